# revision 6
# baseline (speedup 1.0000x reference)
"""LIF neuron step on 8 Trainium2 NeuronCores.

Math (reference):
    I_raw   = g @ w                       # [N] vec-mat product, w is [N, N]
    I       = sigmoid(12/N * I_raw) + 0.9 * x_in
    v_next  = v + (E_L - v + I * (30 - E_L)) / tau_m
    out     = sigmoid(v_next - 30)

Everything after the matvec is affine in I_sig = sigmoid(12/N * I_raw):
    out = sigmoid(B * (I_sig + D/B)),  B = (30 - E_L)/tau_m (uniform scalar),
    D   = v + (E_L - v)/tau_m - 30 + 0.9 * x_in * B  (per-neuron, host-computed)

Sharding: w is split column-wise (output-neuron dim) into 8 shards of
[8192, 1024]; g is replicated. Each core computes its 1024 outputs fully
locally; host concatenates.

The kernel is memory-bound on streaming the w shard, so w/g are cast to
fp8 e4m3 on the host (the matvec averages 8192 near-independent products
so quantization noise mostly cancels; measured output rel-err ~9e-3 vs
the 2e-2 gate).  The host pre-arranges the shard in the exact SBUF layout
([p, t, c] slots, one k-row per slot) so every chunk DMA is a contiguous
run per partition.

PE structure: g is the STATIONARY operand ([*, 2, 1] fp8 pairs) and w the
MOVING operand ([*, 2, 512] slabs) with perf_mode=DoubleRow (256 moving
elements/cycle).  Only ~140 PE instructions total, so the NEFF
instruction stream fits one iram page (per-tile LDWEIGHTS needs ~1000 and
the resulting demand paging rides DMA engine 0 mid-stream).

DMA engine 15 (the one serving partitions {92-95,124-127}) runs ~20%
slower than the rest (known TRN2 quirk), and every chunk-completion
semaphore waits for the slowest engine.  The k->slot assignment is
therefore rebalanced: those 8 partitions only carry k-slots for t<52
(52 KB each) and the displaced 1536 k-rows ride 14 extra t-slots on the
other 120 partitions (66 KB each), streamed as two extra partition-range
DMAs + matmuls (partition groups [0:92) and [96:124)).  All 16 engines
then finish within ~1 us of each other.

The per-core result accumulates as two [1, 512] fp32 PSUM tiles; the
tail is sigmoid/add/sigmoid in half-slabs pipelined across ACT and DVE,
then two single-descriptor 2 KB output DMAs.
"""

from contextlib import ExitStack

import numpy as np

import concourse.bass as bass
import concourse.bacc as bacc
import concourse.mybir as mybir
import concourse.tile as tile
from concourse.bass_utils import run_bass_kernel_spmd

N = 8192          # neurons
NCORES = 8
COLS = N // NCORES  # 1024 output neurons per core
P = 128           # partitions
TA = 52           # k-tiles carried by ALL partitions (region A)
T = 66            # total k-tile slots (region B: t in [52,66) on fast parts)
SKTA = TA // 2    # 26 super k-tiles in region A
SKT = T // 2      # 33 total super k-tiles
# region-A chunk sizes in super-ktiles: small first chunk so PE starts
# early, small last chunk so PE finishes right behind the final packet.
CHUNK_SIZES = [2, 6, 8, 6, 3, 1]
SLOW = (92, 96, 124, 128)   # partition ranges [92:96) and [124:128) (engine 15)
GPAD = 16         # stationary dim-1 step must be 16B-aligned
SPIKE = 30.0
F8 = mybir.dt.float8e4

TRACE = False          # set True to capture NTFF profile
LAST_RESULT = None     # BassKernelResults of the most recent run

_NC = None
_B_CONST = None


def _build(b_const):
    nc = bacc.Bacc("TRN2", target_bir_lowering=False, debug=False,
                   num_devices=NCORES)
    # host pre-layout: wt[p, t*COLS + c] = w[k(p, t), c]  (fp8)
    wt = nc.dram_tensor("wt", [P, T * COLS], F8, kind="ExternalInput").ap()
    # gt[p, t*GPAD] = g[k(p, t)], zero-padded so the DoubleRow stationary
    # AP's middle-dim step is 16 bytes.
    gt = nc.dram_tensor("gt", [P, T * GPAD], F8, kind="ExternalInput").ap()
    db = nc.dram_tensor("db", [1, COLS], mybir.dt.float32,
                        kind="ExternalInput").ap()
    out = nc.dram_tensor("out", [1, COLS], mybir.dt.float32,
                         kind="ExternalOutput").ap()

    with tile.TileContext(nc) as tc, ExitStack() as ctx:
        wpool = ctx.enter_context(tc.tile_pool(name="w", bufs=1))
        spool = ctx.enter_context(tc.tile_pool(name="s", bufs=1))
        ppool = ctx.enter_context(tc.tile_pool(name="p", bufs=1, space="PSUM"))

        # region-A chunks stream on the SP HWDGE ring.  The two region-B
        # rectangles are issued right after the first small chunk so their
        # matmuls can run mid-stream; DMA engine 15 has no region-B bytes
        # and drains its own ring independently.
        awsbs = []
        s0 = 0
        for ci, sct in enumerate(CHUNK_SIZES):
            ct = 2 * sct
            wsb = wpool.tile([P, ct * COLS], F8, tag=f"w{ci}")
            nc.sync.dma_start(wsb[:], wt[:, 2 * s0 * COLS:
                                         2 * (s0 + sct) * COLS])
            awsbs.append((s0, sct, wsb))
            s0 += sct
            if ci == 0:
                tb = T - TA
                bw1 = wpool.tile([92, tb * COLS], F8, tag="bw1")
                nc.sync.dma_start(bw1[:], wt[0:92, TA * COLS:T * COLS])
                bw2 = wpool.tile([124, tb * COLS], F8, tag="bw2")
                nc.sync.dma_start(bw2[96:124, :],
                                  wt[96:124, TA * COLS:T * COLS])

        gsb = spool.tile([P, T * GPAD], F8)
        nc.scalar.dma_start(gsb[:], gt[:])
        dbsb = spool.tile([1, COLS], mybir.dt.float32)
        nc.scalar.dma_start(dbsb[:], db[:])

        gs3 = gsb[:].rearrange("p (t q) -> p t q", q=GPAD)
        acc = [ppool.tile([1, 512], mybir.dt.float32, tag=f"acc{h}",
                          name=f"acc{h}")
               for h in range(2)]

        def mm(lhsT, rhs, h, start, stop, tile_position=None):
            nc.tensor.matmul(acc[h][:, :], lhsT, rhs, start=start, stop=stop,
                             perf_mode=mybir.MatmulPerfMode.DoubleRow,
                             tile_position=tile_position)

        bws = [bw1[:].rearrange("p (t c) -> p t c", c=COLS),
               bw2[:].rearrange("p (t c) -> p t c", c=COLS)]
        for ci, (s0, sct, wsb) in enumerate(awsbs):
            ws3 = wsb[:].rearrange("p (t c) -> p t c", c=COLS)
            for sl in range(sct):
                s = s0 + sl
                last = (ci == len(awsbs) - 1 and sl == sct - 1)
                for h in range(2):
                    mm(gs3[:, 2 * s:2 * s + 2, 0:1],
                       ws3[:, 2 * sl:2 * sl + 2, 512 * h:512 * (h + 1)],
                       h, start=(s == 0), stop=last)
            if ci == 0:
                # region-B matmuls: partition groups [0:92) and [96:124)
                for sb in range(SKT - SKTA):
                    t = 2 * sb
                    for h in range(2):
                        mm(gs3[0:92, TA + t:TA + t + 2, 0:1],
                           bws[0][0:92, t:t + 2, 512 * h:512 * (h + 1)],
                           h, start=False, stop=False)
                        mm(gs3[96:124, TA + t:TA + t + 2, 0:1],
                           bws[1][96:124, t:t + 2, 512 * h:512 * (h + 1)],
                           h, start=False, stop=False,
                           tile_position=(96, 0))

        # Tail: out = sigmoid(B * (sigmoid(acc*12/N) + D/B)) in half-slabs
        # so DVE adds overlap the ACT sigmoids.
        isig = spool.tile([1, COLS], mybir.dt.float32)
        aff = spool.tile([1, COLS], mybir.dt.float32)
        res = spool.tile([1, COLS], mybir.dt.float32)
        for h in range(2):
            hs = slice(512 * h, 512 * (h + 1))
            nc.scalar.activation(isig[:, hs], acc[h][:, :],
                                 mybir.ActivationFunctionType.Sigmoid,
                                 scale=12.0 / N)
            nc.vector.tensor_add(aff[:, hs], isig[:, hs], dbsb[:, hs])
            nc.scalar.activation(res[:, hs], aff[:, hs],
                                 mybir.ActivationFunctionType.Sigmoid,
                                 scale=float(b_const))
            nc.sync.dma_start(out[:, hs], res[:, hs])
    nc.compile()
    return nc


def _slot_k_map():
    """k-row index for every (p, t) slot; -1 = unused/zero slot."""
    fast = [p for p in range(P)
            if not (SLOW[0] <= p < SLOW[1] or SLOW[2] <= p < SLOW[3])]
    K = np.full((P, T), -1, dtype=np.int64)
    t_idx, p_idx = np.meshgrid(np.arange(TA), np.arange(P), indexing="ij")
    K[:, :TA] = (t_idx * P + p_idx).T
    nxt = TA * P
    for t in range(TA, T):
        for p in fast:
            if nxt < N:
                K[p, t] = nxt
                nxt += 1
    assert nxt == N
    return K


_KMAP = _slot_k_map()


def make_in_maps(x_in, v, g, w, E_L, tau_m, b_const):
    np8 = mybir.dt.np(F8)
    w8 = np.asarray(w, dtype=np.float32).astype(np8)
    g8 = np.asarray(g, dtype=np.float32).astype(np8)

    K = _KMAP
    Ksafe = np.where(K >= 0, K, 0)
    gt = np.zeros((P, T * GPAD), dtype=np8)
    gvals = g8[Ksafe]
    gvals[K < 0] = 0
    gt[:, ::GPAD] = gvals

    E = np.asarray(E_L, dtype=np.float64)
    TM = np.asarray(tau_m, dtype=np.float64)
    V = np.asarray(v, dtype=np.float64)
    X = np.asarray(x_in, dtype=np.float64)
    D = V + (E - V) / TM - SPIKE + 0.9 * X * b_const
    DB = (D / b_const).astype(np.float32)

    in_maps = []
    for c in range(NCORES):
        sl = slice(c * COLS, (c + 1) * COLS)
        wtc = np.ascontiguousarray(
            w8[:, sl][Ksafe.reshape(-1)]).reshape(P, T * COLS)
        in_maps.append({
            "wt": wtc,
            "gt": gt,
            "db": DB[sl].reshape(1, COLS),
        })
    return in_maps


def kernel(x_in, v, g, w, E_L, tau_m, tau_g=None, **_unused):
    global _NC, _B_CONST, LAST_RESULT
    B = (SPIKE - np.asarray(E_L, dtype=np.float64)) \
        / np.asarray(tau_m, dtype=np.float64)
    b_const = float(B[0])
    assert np.allclose(B, b_const, rtol=1e-6), \
        "kernel assumes uniform E_L/tau_m"
    if _NC is None or _B_CONST != b_const:
        _NC = _build(b_const)
        _B_CONST = b_const
    in_maps = make_in_maps(x_in, v, g, w, E_L, tau_m, b_const)
    LAST_RESULT = run_bass_kernel_spmd(_NC, in_maps, list(range(NCORES)),
                                       trace=TRACE)
    out = np.empty(N, dtype=np.float32)
    for c in range(NCORES):
        out[c * COLS:(c + 1) * COLS] = LAST_RESULT.results[c]["out"][0]
    return out


# revision 7
# speedup vs baseline: 2.0158x; 2.0158x over previous
"""LIF neuron step on 8 Trainium2 NeuronCores.

Math (reference):
    I_raw   = g @ w                       # [N] vec-mat product, w is [N, N]
    I       = sigmoid(12/N * I_raw) + 0.9 * x_in
    v_next  = v + (E_L - v + I * (30 - E_L)) / tau_m
    out     = sigmoid(v_next - 30)

Everything after the matvec is affine in I_sig = sigmoid(12/N * I_raw):
    out = sigmoid(B * I_sig + D)
    B   = (30 - E_L) / tau_m
    D   = v + (E_L - v)/tau_m - 30 + 0.9 * x_in * B
B and D are tiny per-neuron vectors, computed on the host.

Sharding: w is split column-wise (output-neuron dim) into 8 shards of
[8192, 1024]; g is replicated. Each core computes its 1024 outputs fully
locally; host concatenates.

The kernel is memory-bound on streaming the w shard, so w/g are cast to
fp8 e4m3 on the host (the matvec is a mean of 8192 near-independent
products, so the quantization noise largely averages out; measured output
rel-err ~9e-3 vs the 2e-2 gate).  The host also pre-arranges the shard
into the exact SBUF layout ([p, t, c] with k = t*128+p) so every chunk
DMA is one contiguous run per partition (128 fat descriptors per chunk,
near line-rate).  PE does the matvec with w-tiles as the stationary
operand (fp8 FWL keeps LDWEIGHTS well ahead of DMA) so the per-core
result lands as a [128, 8] fp32 tile; the elementwise tail is 2 ACT
sigmoids + 2 DVE tensor-tensor ops using all 128 lanes.  Chunk sizes
shrink toward the end: every chunk's matmuls wait on the whole-chunk DMA
completion semaphore (which trails the slowest of the 16 SDMA engines by
1-3 us), so small trailing chunks keep the PE close behind the final
packets.
"""

from contextlib import ExitStack

import numpy as np

import concourse.bass as bass
import concourse.bacc as bacc
import concourse.mybir as mybir
import concourse.tile as tile
from concourse.bass_utils import run_bass_kernel_spmd

N = 8192          # neurons
NCORES = 8
COLS = N // NCORES  # 1024 output neurons per core
P = 128           # partitions
KT = N // P       # 64 contraction tiles
# k-tiles per DMA chunk (sums to KT)
CHUNK_SIZES = [4, 12, 16, 14, 8, 6, 3, 1]
JT = COLS // P    # 8 output tiles per core
SPIKE = 30.0
F8 = mybir.dt.float8e4

TRACE = False          # set True to capture NTFF profile
LAST_RESULT = None     # BassKernelResults of the most recent run

_NC = None


def _build():
    nc = bacc.Bacc("TRN2", target_bir_lowering=False, debug=False,
                   num_devices=NCORES)
    # host pre-layout: wt[p, t*COLS + c] = w[t*128 + p, c]  (fp8)
    wt = nc.dram_tensor("wt", [P, KT * COLS], F8, kind="ExternalInput").ap()
    gt = nc.dram_tensor("gt", [P, KT], F8, kind="ExternalInput").ap()
    bd = nc.dram_tensor("bd", [P, 2 * JT], mybir.dt.float32,
                        kind="ExternalInput").ap()
    out = nc.dram_tensor("out", [P, JT], mybir.dt.float32,
                         kind="ExternalOutput").ap()

    with tile.TileContext(nc) as tc, ExitStack() as ctx:
        wpool = ctx.enter_context(tc.tile_pool(name="w", bufs=1))
        spool = ctx.enter_context(tc.tile_pool(name="s", bufs=1))
        ppool = ctx.enter_context(tc.tile_pool(name="p", bufs=1, space="PSUM"))

        # w chunks stream on the SP HWDGE ring; the small g/bd loads go via
        # the ACT HWDGE ring so their descriptor generation doesn't delay
        # the first w packets.
        wsbs = []
        k0 = 0
        for ci, ct in enumerate(CHUNK_SIZES):
            wsb = wpool.tile([P, ct * COLS], F8, tag=f"w{ci}")
            nc.sync.dma_start(wsb[:], wt[:, k0 * COLS:(k0 + ct) * COLS])
            wsbs.append((k0, ct, wsb))
            k0 += ct

        gsb = spool.tile([P, KT], F8)
        nc.scalar.dma_start(gsb[:], gt[:])
        bdsb = spool.tile([P, 2 * JT], mybir.dt.float32)
        nc.scalar.dma_start(bdsb[:], bd[:])

        acc = ppool.tile([P, JT], mybir.dt.float32)
        for k0, ct, wsb in wsbs:
            for t in range(ct):
                ki = k0 + t
                for jt in range(JT):
                    nc.tensor.matmul(
                        acc[:, jt:jt + 1],
                        wsb[:, t * COLS + jt * P: t * COLS + (jt + 1) * P],
                        gsb[:, ki:ki + 1],
                        start=(ki == 0 and jt == 0),
                        stop=(ki == KT - 1 and jt == JT - 1),
                    )

        # Tail: out = sigmoid(B * sigmoid(acc*12/N) + D).  B/D vary per
        # output neuron (partition AND free dim), so the affine step runs
        # on DVE as two tensor-tensor ops; the sigmoids are single ACT ops
        # over the whole [128, 8] tile.
        isig = spool.tile([P, JT], mybir.dt.float32)
        nc.scalar.activation(isig[:], acc[:],
                             mybir.ActivationFunctionType.Sigmoid,
                             scale=12.0 / N)
        aff = spool.tile([P, JT], mybir.dt.float32)
        nc.vector.tensor_mul(aff[:], isig[:], bdsb[:, 0:JT])
        aff2 = spool.tile([P, JT], mybir.dt.float32)
        nc.vector.tensor_add(aff2[:], aff[:], bdsb[:, JT:2 * JT])
        res = spool.tile([P, JT], mybir.dt.float32)
        nc.scalar.activation(res[:], aff2[:],
                             mybir.ActivationFunctionType.Sigmoid)
        nc.sync.dma_start(out[:], res[:])
    nc.compile()
    return nc


def make_in_maps(x_in, v, g, w, E_L, tau_m):
    np8 = mybir.dt.np(F8)
    w8 = np.asarray(w, dtype=np.float32).astype(np8)
    g8t = np.ascontiguousarray(
        np.asarray(g, dtype=np.float32).astype(np8).reshape(KT, P).T)

    E = np.asarray(E_L, dtype=np.float64)
    TM = np.asarray(tau_m, dtype=np.float64)
    V = np.asarray(v, dtype=np.float64)
    X = np.asarray(x_in, dtype=np.float64)
    B = (SPIKE - E) / TM
    D = V + (E - V) / TM - SPIKE + 0.9 * X * B

    in_maps = []
    for c in range(NCORES):
        sl = slice(c * COLS, (c + 1) * COLS)
        # [p, t, c] layout: partition p holds k-tiles t=0..KT-1 contiguously
        wtc = np.ascontiguousarray(
            w8[:, sl].reshape(KT, P, COLS).transpose(1, 0, 2)
        ).reshape(P, KT * COLS)
        bdc = np.concatenate(
            [B[sl].astype(np.float32).reshape(JT, P).T,
             D[sl].astype(np.float32).reshape(JT, P).T], axis=1)
        in_maps.append({
            "wt": wtc,
            "gt": g8t,
            "bd": np.ascontiguousarray(bdc),
        })
    return in_maps


def kernel(x_in, v, g, w, E_L, tau_m, tau_g=None, **_unused):
    global _NC, LAST_RESULT
    if _NC is None:
        _NC = _build()
    in_maps = make_in_maps(x_in, v, g, w, E_L, tau_m)
    LAST_RESULT = run_bass_kernel_spmd(_NC, in_maps, list(range(NCORES)),
                                       trace=TRACE)
    out = np.empty(N, dtype=np.float32)
    for c in range(NCORES):
        out[c * COLS:(c + 1) * COLS] = \
            LAST_RESULT.results[c]["out"].T.reshape(COLS)
    return out
